# revision 11
# baseline (speedup 1.0000x reference)
import sys

sys.path.insert(0, "/opt/trn_rl_repo")

import numpy as np
import ml_dtypes

BF16 = ml_dtypes.bfloat16
T, B, D_IN, D_DAY, H, D_OUT = 64, 32, 4096, 256, 256, 942
NC = 8
BL = B // NC          # batch lanes per core
NB = T * BL           # (row, b) pairs per core = 256


def _sigmoid(x):
    with np.errstate(over="ignore"):
        return 1.0 / (1.0 + np.exp(-x))


def _host_fwd(day_emb, Wih, Whh, bih, bhh):
    # full forward GRU on host (tiny): day_emb [T,B,D_DAY] -> fwd [T,B,H]
    f32 = np.float32
    gi = day_emb.reshape(T * B, D_DAY) @ Wih.T.astype(f32) + bih
    gi = gi.reshape(T, B, 3 * H)
    WhhT = Whh.T.astype(f32)
    fwd = np.empty((T, B, H), f32)
    h = np.zeros((B, H), f32)
    for t in range(T):
        gh = h @ WhhT + bhh
        ir, iz, inn = gi[t, :, :H], gi[t, :, H:2 * H], gi[t, :, 2 * H:]
        hr, hz, hn = gh[:, :H], gh[:, H:2 * H], gh[:, 2 * H:]
        r = _sigmoid(ir + hr)
        z = _sigmoid(iz + hz)
        n = np.tanh(inn + r * hn)
        h = (1.0 - z) * n + z * h
        fwd[t] = h
    return fwd


def _fold2(v):  # [256] -> [128, 2]  (p, k) with feature = k*128+p
    return np.ascontiguousarray(v.reshape(2, 128).T)


_NC_CACHE = {}


def _build_nc():
    if "nc" in _NC_CACHE:
        return _NC_CACHE["nc"]
    from contextlib import ExitStack
    import concourse.bass as bass
    from concourse import bacc, mybir
    from concourse.tile import TileContext

    bf = mybir.dt.bfloat16
    f32 = mybir.dt.float32
    AF = mybir.ActivationFunctionType
    ALU = mybir.AluOpType

    nc = bacc.Bacc("TRN2", num_devices=NC, debug=False, enable_asserts=False)

    def din(name, shape, dt):
        return nc.dram_tensor(name, shape, dt, kind="ExternalInput").ap()

    embT = din("embT", [128, 2, T, BL], bf)
    fwdT = din("fwdT", [128, 2, T, BL], bf)
    wih = din("wih", [128, 2, 6, 128], bf)
    whh = din("whh", [128, 2, 6, 128], bf)
    wr = din("wr", [128, 2], bf)
    ones1 = din("ones1", [1, 128], bf)
    fvec = din("fvec", [1, T, BL], f32)
    brz = din("brz", [128, 4], f32)
    bnih = din("bnih", [128, 2], f32)
    bnhh = din("bnhh", [128, 2], f32)
    waot = din("waot", [128, 8, 2, 128], bf)
    bao = din("bao", [128, 2], f32)
    wot = din("wot", [128, 2, D_OUT], bf)
    bo = din("bo", [1, D_OUT], bf)
    icnt = din("icnt", [1, T, BL], f32)
    out_d = nc.dram_tensor("out", [NB, D_OUT], f32, kind="ExternalOutput").ap()

    with TileContext(nc) as tc, ExitStack() as ctx:
        cpool = ctx.enter_context(tc.tile_pool(name="consts", bufs=1))

        def load(ap_dram, shape, dt):
            t = cpool.tile(shape, dt, tag=ap_dram.tensor.name)
            nc.sync.dma_start(t[:], ap_dram)
            return t

        emb_s = load(embT, [128, 2, T, BL], bf)
        fwd_s = load(fwdT, [128, 2, T, BL], bf)
        wih_s = load(wih, [128, 2, 6, 128], bf)
        whh_s = load(whh, [128, 2, 6, 128], bf)
        wr_s = load(wr, [128, 2], bf)
        on_s = load(ones1, [1, 128], bf)
        fv_s = load(fvec, [1, T, BL], f32)
        brz_s = load(brz, [128, 4], f32)
        bnih_s = load(bnih, [128, 2], f32)
        bnhh_s = load(bnhh, [128, 2], f32)
        waot_s = load(waot, [128, 8, 2, 128], bf)
        bao_s = load(bao, [128, 2], f32)
        wot_s = load(wot, [128, 2, D_OUT], bf)
        bo_s = load(bo, [1, D_OUT], bf)
        icnt_s = load(icnt, [1, T, BL], f32)

        # persistent state / accumulators
        spool = ctx.enter_context(tc.tile_pool(name="state", bufs=1))
        state = spool.tile([128, 2, T, BL], bf, tag="state")
        gix = spool.tile([128, 6, T, BL], bf, tag="gix")
        accR = spool.tile([128, 2, T, BL], f32, tag="accR")
        accF = spool.tile([128, 2, T, BL], f32, tag="accF")
        dacc = spool.tile([1, T, BL], f32, tag="dacc")
        h_t = spool.tile([128, 8, T, BL], bf, tag="h_t")

        nc.vector.memset(state[:], 0.0)
        nc.vector.memset(accR[:], 0.0)
        nc.vector.memset(accF[:], 0.0)
        nc.vector.memset(dacc[:], 0.0)

        # ---- bulk input projection: gix = Wih_r @ embT (+bih for n gates) ----
        with tc.tile_pool(name="pgix", bufs=1, space="PSUM") as pgix:
            pg = pgix.tile([128, 6, T, BL], f32)
            for m in range(6):
                for k in range(2):
                    nc.tensor.matmul(
                        pg[:, m], wih_s[:, k, m], emb_s[:, k],
                        start=(k == 0), stop=(k == 1),
                    )
            # rz gates: no bias here (added at sigmoid); n gates: + bih_n
            nc.scalar.activation(gix[:, 0:4], pg[:, 0:4], AF.Identity)
            for k in range(2):
                nc.scalar.activation(
                    gix[:, 4 + k], pg[:, 4 + k], AF.Identity,
                    bias=bnih_s[:, k:k + 1],
                )

        # ---- reverse recurrence with online attention ----
        lpool = ctx.enter_context(tc.tile_pool(name="loop", bufs=2))
        lctx = ctx.enter_context(ExitStack())
        prz_p = lctx.enter_context(tc.tile_pool(name="prz", bufs=1, space="PSUM"))
        phn_p = lctx.enter_context(tc.tile_pool(name="phn", bufs=1, space="PSUM"))
        ps_p = lctx.enter_context(tc.tile_pool(name="ps", bufs=2, space="PSUM"))
        ppb_p = lctx.enter_context(tc.tile_pool(name="ppb", bufs=2, space="PSUM"))

        for j in range(T):
            nact = T - j
            prz = prz_p.tile([128, 4, T, BL], f32, tag="prz")
            phn = phn_p.tile([128, 2, T, BL], f32, tag="phn")
            for m in range(4):
                for k in range(2):
                    nc.tensor.matmul(
                        prz[:, m, :nact], whh_s[:, k, m], state[:, k, j:],
                        start=(k == 0), stop=(k == 1),
                    )
            for m in range(2):
                for k in range(2):
                    nc.tensor.matmul(
                        phn[:, m, :nact], whh_s[:, k, 4 + m], state[:, k, j:],
                        start=(k == 0), stop=(k == 1),
                    )
            trz = lpool.tile([128, 4, T, BL], f32, tag="trz")
            nc.vector.tensor_add(trz[:, :, :nact], prz[:, :, :nact], gix[:, 0:4, :nact])
            rz = lpool.tile([128, 4, T, BL], bf, tag="rz")
            for m in range(4):
                nc.scalar.activation(
                    rz[:, m, :nact], trz[:, m, :nact], AF.Sigmoid,
                    bias=brz_s[:, m:m + 1],
                )
            # rn = (hn + bhh_n) * r  per fold
            rn = lpool.tile([128, 2, T, BL], f32, tag="rn")
            for k in range(2):
                nc.vector.scalar_tensor_tensor(
                    rn[:, k, :nact], phn[:, k, :nact], bnhh_s[:, k:k + 1],
                    rz[:, k, :nact], ALU.add, ALU.mult,
                )
            narg = lpool.tile([128, 2, T, BL], f32, tag="narg")
            nc.vector.tensor_add(narg[:, :, :nact], rn[:, :, :nact], gix[:, 4:6, :nact])
            n_g = lpool.tile([128, 2, T, BL], bf, tag="n_g")
            nc.scalar.activation(n_g[:, :, :nact], narg[:, :, :nact], AF.Tanh)
            dmn = lpool.tile([128, 2, T, BL], bf, tag="dmn")
            nc.vector.tensor_sub(dmn[:, :, :nact], state[:, :, j:], n_g[:, :, :nact])
            ezd = lpool.tile([128, 2, T, BL], bf, tag="ezd")
            nc.vector.tensor_mul(ezd[:, :, :nact], rz[:, 2:4, :nact], dmn[:, :, :nact])
            nc.vector.tensor_add(state[:, :, j:], n_g[:, :, :nact], ezd[:, :, :nact])

            # attention: s = w_r . state_new + f_j + attn_b ; p = exp(s)
            ps = ps_p.tile([1, T, BL], f32, tag="ps")
            for k in range(2):
                nc.tensor.matmul(
                    ps[:, :nact], wr_s[:, k:k + 1], state[:, k, j:],
                    start=(k == 0), stop=(k == 1),
                )
            s_f = lpool.tile([1, T, BL], f32, tag="s_f")
            nc.vector.tensor_add(
                s_f[:, :nact], ps[:, :nact],
                fv_s[:, j:j + 1].broadcast_to([1, nact, BL]),
            )
            p_b = lpool.tile([1, T, BL], bf, tag="p_b")
            nc.scalar.activation(p_b[:, :nact], s_f[:, :nact], AF.Exp)
            nc.vector.tensor_add(dacc[:, j:], dacc[:, j:], p_b[:, :nact])
            ppb = ppb_p.tile([128, T, BL], f32, tag="ppb")
            nc.tensor.matmul(ppb[:, :nact], on_s[:1], p_b[:, :nact],
                             start=True, stop=True)
            tmr = lpool.tile([128, 2, T, BL], f32, tag="tmr")
            tmf = lpool.tile([128, 2, T, BL], f32, tag="tmf")
            for k in range(2):
                nc.vector.tensor_mul(tmr[:, k, :nact], ppb[:, :nact], state[:, k, j:])
                nc.vector.tensor_add(accR[:, k, j:], accR[:, k, j:], tmr[:, k, :nact])
                nc.vector.tensor_mul(
                    tmf[:, k, :nact], ppb[:, :nact],
                    fwd_s[:, k, j:j + 1].broadcast_to([128, nact, BL]),
                )
                nc.vector.tensor_add(accF[:, k, j:], accF[:, k, j:], tmf[:, k, :nact])

        # ---- epilogue: normalize, assemble h_t, two projections, DMA out ----
        lctx.close()
        epool = ctx.enter_context(tc.tile_pool(name="epi", bufs=1))
        invd = epool.tile([1, T, BL], f32, tag="invd")
        nc.vector.reciprocal(invd[:], dacc[:])
        scal = epool.tile([1, T, BL], bf, tag="scal")
        nc.vector.tensor_mul(scal[:], invd[:], icnt_s[:])
        with tc.tile_pool(name="psc", bufs=1, space="PSUM") as psc_p:
            psc = psc_p.tile([128, T, BL], f32)
            nc.tensor.matmul(psc[:], on_s[:1], scal[:], start=True, stop=True)
            for k in range(2):
                nc.vector.tensor_mul(h_t[:, 0 + k], accF[:, k], psc[:])
                nc.vector.tensor_mul(h_t[:, 2 + k], accR[:, k], psc[:])
                nc.vector.tensor_copy(h_t[:, 4 + k], fwd_s[:, k])
                nc.vector.tensor_copy(h_t[:, 6 + k], state[:, k])
        hto = epool.tile([128, 2, T, BL], bf, tag="hto")
        with tc.tile_pool(name="pao", bufs=1, space="PSUM") as pao_p:
            pao = pao_p.tile([128, 2, T, BL], f32)
            for m in range(2):
                for k in range(8):
                    nc.tensor.matmul(
                        pao[:, m], waot_s[:, k, m], h_t[:, k],
                        start=(k == 0), stop=(k == 7),
                    )
                nc.scalar.activation(hto[:, m], pao[:, m], AF.Identity,
                                     bias=bao_s[:, m:m + 1])
        outT = epool.tile([128, 2, D_OUT], f32, tag="outT")
        with tc.tile_pool(name="pout", bufs=1, space="PSUM") as pout_p:
            for m in range(2):
                po = pout_p.tile([128, 1024], f32, tag="po")
                for c0, c1 in ((0, 512), (512, D_OUT)):
                    for k in range(2):
                        nc.tensor.matmul(
                            po[:, c0:c1],
                            hto[:, k, m * 32:(m + 1) * 32], wot_s[:, k, c0:c1],
                            start=(k == 0), stop=False,
                        )
                    nc.tensor.matmul(po[:, c0:c1], on_s[:1], bo_s[:, c0:c1],
                                     start=False, stop=True)
                nc.scalar.activation(outT[:, m], po[:, :D_OUT], AF.Sigmoid)
                nc.sync.dma_start(out_d[m * 128:(m + 1) * 128], outT[:, m])

    nc.finalize()
    _NC_CACHE["nc"] = nc
    return nc


def kernel(**inputs):
    f32 = np.float32
    x = np.asarray(inputs["x"], f32)
    W_emb = np.asarray(inputs["W_emb"], f32)
    b_emb = np.asarray(inputs["b_emb"], f32)
    attn_w = np.asarray(inputs["attn_w"], f32)
    attn_b = float(np.asarray(inputs["attn_b"], f32))

    day_emb = (x.reshape(T * B, D_IN) @ W_emb.T + b_emb).reshape(T, B, D_DAY)
    fwd = _host_fwd(day_emb, inputs["Wih_f"], inputs["Whh_f"],
                    np.asarray(inputs["bih_f"], f32), np.asarray(inputs["bhh_f"], f32))
    w_f, w_r = attn_w[:H], attn_w[H:]
    fsc = fwd @ w_f + attn_b  # [T, B], attn_b folded in


    Wih_r = np.asarray(inputs["Wih_r"], f32)
    Whh_r = np.asarray(inputs["Whh_r"], f32)
    bih_r = np.asarray(inputs["bih_r"], f32)
    bhh_r = np.asarray(inputs["bhh_r"], f32)
    W_ao = np.asarray(inputs["W_ao"], f32)
    b_ao = np.asarray(inputs["b_ao"], f32)
    W_o = np.asarray(inputs["W_o"], f32)
    b_o = np.asarray(inputs["b_o"], f32)

    # shared (replicated) tensors
    wih_h = np.ascontiguousarray(
        Wih_r.reshape(6, 128, 2, 128).transpose(3, 2, 0, 1)).astype(BF16)
    whh_h = np.ascontiguousarray(
        Whh_r.reshape(6, 128, 2, 128).transpose(3, 2, 0, 1)).astype(BF16)
    wr_h = _fold2(w_r).astype(BF16)
    ones_h = np.ones((1, 128), BF16)
    brz_h = np.ascontiguousarray(
        (bih_r[:512] + bhh_r[:512]).reshape(4, 128).T).astype(f32)
    bnih_h = _fold2(bih_r[512:]).astype(f32)
    bnhh_h = _fold2(bhh_r[512:]).astype(f32)
    waot_h = np.ascontiguousarray(
        W_ao.reshape(2, 128, 8, 128).transpose(3, 2, 0, 1)).astype(BF16)
    bao_h = _fold2(b_ao).astype(f32)
    wot_h = np.ascontiguousarray(
        W_o.reshape(D_OUT, 2, 128).transpose(2, 1, 0)).astype(BF16)
    bo_h = b_o.reshape(1, D_OUT).astype(BF16)
    icnt_h = np.broadcast_to(
        (1.0 / np.arange(1, T + 1, dtype=f32))[:, None], (T, BL)
    ).reshape(1, T, BL).astype(f32)

    embT_all = np.ascontiguousarray(
        day_emb.transpose(2, 0, 1).reshape(2, 128, T, B).transpose(1, 0, 2, 3)
    ).astype(BF16)  # [128, 2, T, B]
    fwdT_all = np.ascontiguousarray(
        fwd.transpose(2, 0, 1).reshape(2, 128, T, B).transpose(1, 0, 2, 3)
    ).astype(BF16)

    in_maps = []
    for c in range(NC):
        bs = slice(c * BL, (c + 1) * BL)
        in_maps.append({
            "embT": np.ascontiguousarray(embT_all[:, :, :, bs]),
            "fwdT": np.ascontiguousarray(fwdT_all[:, :, :, bs]),
            "wih": wih_h, "whh": whh_h, "wr": wr_h, "ones1": ones_h,
            "fvec": np.ascontiguousarray(fsc[:, bs]).reshape(1, T, BL).astype(f32),
            "brz": brz_h, "bnih": bnih_h, "bnhh": bnhh_h,
            "waot": waot_h, "bao": bao_h, "wot": wot_h, "bo": bo_h,
            "icnt": icnt_h,
        })

    nc = _build_nc()
    from concourse.bass_utils import run_bass_kernel_spmd
    res = run_bass_kernel_spmd(nc, in_maps, core_ids=list(range(NC)))

    out = np.empty((T, B, D_OUT), f32)
    for c in range(NC):
        out[:, c * BL:(c + 1) * BL, :] = res.results[c]["out"].reshape(T, BL, D_OUT)
    return out


# revision 14
# speedup vs baseline: 1.1581x; 1.1581x over previous
import sys

sys.path.insert(0, "/opt/trn_rl_repo")

import numpy as np
import ml_dtypes

BF16 = ml_dtypes.bfloat16
T, B, D_IN, D_DAY, H, D_OUT = 64, 32, 4096, 256, 256, 942
NC = 8
BL = B // NC          # batch lanes per core
NB = T * BL           # (row, b) pairs per core = 256


def _sigmoid(x):
    with np.errstate(over="ignore"):
        return 1.0 / (1.0 + np.exp(-x))


def _host_fwd(day_emb, Wih, Whh, bih, bhh):
    # full forward GRU on host (tiny): day_emb [T,B,D_DAY] -> fwd [T,B,H]
    f32 = np.float32
    gi = day_emb.reshape(T * B, D_DAY) @ Wih.T.astype(f32) + bih
    gi = gi.reshape(T, B, 3 * H)
    WhhT = Whh.T.astype(f32)
    fwd = np.empty((T, B, H), f32)
    h = np.zeros((B, H), f32)
    for t in range(T):
        gh = h @ WhhT + bhh
        ir, iz, inn = gi[t, :, :H], gi[t, :, H:2 * H], gi[t, :, 2 * H:]
        hr, hz, hn = gh[:, :H], gh[:, H:2 * H], gh[:, 2 * H:]
        r = _sigmoid(ir + hr)
        z = _sigmoid(iz + hz)
        n = np.tanh(inn + r * hn)
        h = (1.0 - z) * n + z * h
        fwd[t] = h
    return fwd


def _fold2(v):  # [256] -> [128, 2]  (p, k) with feature = k*128+p
    return np.ascontiguousarray(v.reshape(2, 128).T)


_NC_CACHE = {}


def _build_nc(consts):
    if "nc" in _NC_CACHE:
        return _NC_CACHE["nc"]
    from contextlib import ExitStack
    import concourse.bass as bass
    from concourse import bacc, mybir
    from concourse.tile import TileContext

    bf = mybir.dt.bfloat16
    f32 = mybir.dt.float32
    AF = mybir.ActivationFunctionType
    ALU = mybir.AluOpType

    nc = bacc.Bacc("TRN2", num_devices=NC, debug=False, enable_asserts=False)

    def din(name, shape, dt):
        return nc.dram_tensor(name, shape, dt, kind="ExternalInput").ap()

    embT = din("embT", [128, 2, T, BL], bf)
    fwdT = din("fwdT", [128, 2, T, BL], bf)
    fvec = din("fvec", [1, T, BL], f32)

    def cin(name):
        return nc.inline_tensor(consts[name], name=name).ap()

    wih, whh, wr, ones1 = cin("wih"), cin("whh"), cin("wr"), cin("ones1")
    brz, bnih, bnhh = cin("brz"), cin("bnih"), cin("bnhh")
    waot, bao, wot, bo, icnt = (cin("waot"), cin("bao"), cin("wot"),
                                cin("bo"), cin("icnt"))
    out_d = nc.dram_tensor("out", [NB, D_OUT], f32, kind="ExternalOutput").ap()

    with TileContext(nc) as tc, ExitStack() as ctx:
        cpool = ctx.enter_context(tc.tile_pool(name="consts", bufs=1))

        def load(ap_dram, shape, dt):
            t = cpool.tile(shape, dt, tag=ap_dram.tensor.name)
            nc.sync.dma_start(t[:], ap_dram)
            return t

        emb_s = load(embT, [128, 2, T, BL], bf)
        fwd_s = load(fwdT, [128, 2, T, BL], bf)
        wih_s = load(wih, [128, 2, 6, 128], bf)
        whh_s = load(whh, [128, 2, 6, 128], bf)
        wr_s = load(wr, [128, 2], bf)
        on_s = load(ones1, [1, 128], bf)
        fv_s = load(fvec, [1, T, BL], f32)
        brz_s = load(brz, [128, 4], f32)
        bnih_s = load(bnih, [128, 2], f32)
        bnhh_s = load(bnhh, [128, 2], f32)
        waot_s = load(waot, [128, 8, 2, 128], bf)
        bao_s = load(bao, [128, 2], f32)
        wot_s = load(wot, [128, 2, D_OUT], bf)
        bo_s = load(bo, [1, D_OUT], bf)
        icnt_s = load(icnt, [1, T, BL], f32)

        # persistent state / accumulators
        spool = ctx.enter_context(tc.tile_pool(name="state", bufs=1))
        state = spool.tile([128, 2, T, BL], bf, tag="state")
        gix = spool.tile([128, 6, T, BL], bf, tag="gix")
        accR = spool.tile([128, 2, T, BL], f32, tag="accR")
        accF = spool.tile([128, 2, T, BL], f32, tag="accF")
        dacc = spool.tile([1, T, BL], f32, tag="dacc")
        h_t = spool.tile([128, 8, T, BL], bf, tag="h_t")

        nc.vector.memset(state[:], 0.0)
        nc.vector.memset(accR[:], 0.0)
        nc.vector.memset(accF[:], 0.0)
        nc.vector.memset(dacc[:], 0.0)

        # ---- bulk input projection: gix = Wih_r @ embT (+bih for n gates) ----
        with tc.tile_pool(name="pgix", bufs=1, space="PSUM") as pgix:
            pg = pgix.tile([128, 6, T, BL], f32)
            for m in range(6):
                for k in range(2):
                    nc.tensor.matmul(
                        pg[:, m], wih_s[:, k, m], emb_s[:, k],
                        start=(k == 0), stop=(k == 1),
                    )
            # rz gates: no bias here (added at sigmoid); n gates: + bih_n
            nc.scalar.activation(gix[:, 0:4], pg[:, 0:4], AF.Identity)
            for k in range(2):
                nc.scalar.activation(
                    gix[:, 4 + k], pg[:, 4 + k], AF.Identity,
                    bias=bnih_s[:, k:k + 1],
                )

        # ---- reverse recurrence with online attention ----
        lpool = ctx.enter_context(tc.tile_pool(name="loop", bufs=2))
        lctx = ctx.enter_context(ExitStack())
        prz_p = lctx.enter_context(tc.tile_pool(name="prz", bufs=1, space="PSUM"))
        phn_p = lctx.enter_context(tc.tile_pool(name="phn", bufs=1, space="PSUM"))
        ps_p = lctx.enter_context(tc.tile_pool(name="ps", bufs=2, space="PSUM"))
        ppb_p = lctx.enter_context(tc.tile_pool(name="ppb", bufs=2, space="PSUM"))

        for j in range(T):
            nact = T - j
            prz = prz_p.tile([128, 4, T, BL], f32, tag="prz")
            phn = phn_p.tile([128, 2, T, BL], f32, tag="phn")
            for m in range(4):
                for k in range(2):
                    nc.tensor.matmul(
                        prz[:, m, :nact], whh_s[:, k, m], state[:, k, j:],
                        start=(k == 0), stop=(k == 1),
                    )
            for m in range(2):
                for k in range(2):
                    nc.tensor.matmul(
                        phn[:, m, :nact], whh_s[:, k, 4 + m], state[:, k, j:],
                        start=(k == 0), stop=(k == 1),
                    )
            trz = lpool.tile([128, 4, T, BL], f32, tag="trz")
            nc.vector.tensor_add(trz[:, :, :nact], prz[:, :, :nact], gix[:, 0:4, :nact])
            rz = lpool.tile([128, 4, T, BL], bf, tag="rz")
            for m in range(4):
                nc.scalar.activation(
                    rz[:, m, :nact], trz[:, m, :nact], AF.Sigmoid,
                    bias=brz_s[:, m:m + 1],
                )
            # rn = (hn + bhh_n) * r  per fold
            rn = lpool.tile([128, 2, T, BL], f32, tag="rn")
            for k in range(2):
                nc.vector.scalar_tensor_tensor(
                    rn[:, k, :nact], phn[:, k, :nact], bnhh_s[:, k:k + 1],
                    rz[:, k, :nact], ALU.add, ALU.mult,
                )
            narg = lpool.tile([128, 2, T, BL], f32, tag="narg")
            nc.vector.tensor_add(narg[:, :, :nact], rn[:, :, :nact], gix[:, 4:6, :nact])
            n_g = lpool.tile([128, 2, T, BL], bf, tag="n_g")
            nc.scalar.activation(n_g[:, :, :nact], narg[:, :, :nact], AF.Tanh)
            dmn = lpool.tile([128, 2, T, BL], bf, tag="dmn")
            nc.vector.tensor_sub(dmn[:, :, :nact], state[:, :, j:], n_g[:, :, :nact])
            ezd = lpool.tile([128, 2, T, BL], bf, tag="ezd")
            nc.vector.tensor_mul(ezd[:, :, :nact], rz[:, 2:4, :nact], dmn[:, :, :nact])
            nc.vector.tensor_add(state[:, :, j:], n_g[:, :, :nact], ezd[:, :, :nact])

            # attention: s = w_r . state_new + f_j + attn_b ; p = exp(s)
            ps = ps_p.tile([1, T, BL], f32, tag="ps")
            for k in range(2):
                nc.tensor.matmul(
                    ps[:, :nact], wr_s[:, k:k + 1], state[:, k, j:],
                    start=(k == 0), stop=(k == 1),
                )
            s_f = lpool.tile([1, T, BL], f32, tag="s_f")
            nc.vector.tensor_add(
                s_f[:, :nact], ps[:, :nact],
                fv_s[:, j:j + 1].broadcast_to([1, nact, BL]),
            )
            p_b = lpool.tile([1, T, BL], bf, tag="p_b")
            nc.scalar.activation(p_b[:, :nact], s_f[:, :nact], AF.Exp)
            nc.vector.tensor_add(dacc[:, j:], dacc[:, j:], p_b[:, :nact])
            ppb = ppb_p.tile([128, T, BL], f32, tag="ppb")
            nc.tensor.matmul(ppb[:, :nact], on_s[:1], p_b[:, :nact],
                             start=True, stop=True)
            tmr = lpool.tile([128, 2, T, BL], f32, tag="tmr")
            tmf = lpool.tile([128, 2, T, BL], f32, tag="tmf")
            for k in range(2):
                nc.vector.tensor_mul(tmr[:, k, :nact], ppb[:, :nact], state[:, k, j:])
                nc.vector.tensor_add(accR[:, k, j:], accR[:, k, j:], tmr[:, k, :nact])
                nc.vector.tensor_mul(
                    tmf[:, k, :nact], ppb[:, :nact],
                    fwd_s[:, k, j:j + 1].broadcast_to([128, nact, BL]),
                )
                nc.vector.tensor_add(accF[:, k, j:], accF[:, k, j:], tmf[:, k, :nact])

        # ---- epilogue: normalize, assemble h_t, two projections, DMA out ----
        lctx.close()
        epool = ctx.enter_context(tc.tile_pool(name="epi", bufs=1))
        invd = epool.tile([1, T, BL], f32, tag="invd")
        nc.vector.reciprocal(invd[:], dacc[:])
        scal = epool.tile([1, T, BL], bf, tag="scal")
        nc.vector.tensor_mul(scal[:], invd[:], icnt_s[:])
        with tc.tile_pool(name="psc", bufs=1, space="PSUM") as psc_p:
            psc = psc_p.tile([128, T, BL], f32)
            nc.tensor.matmul(psc[:], on_s[:1], scal[:], start=True, stop=True)
            for k in range(2):
                nc.vector.tensor_mul(h_t[:, 0 + k], accF[:, k], psc[:])
                nc.vector.tensor_mul(h_t[:, 2 + k], accR[:, k], psc[:])
                nc.vector.tensor_copy(h_t[:, 4 + k], fwd_s[:, k])
                nc.vector.tensor_copy(h_t[:, 6 + k], state[:, k])
        hto = epool.tile([128, 2, T, BL], bf, tag="hto")
        with tc.tile_pool(name="pao", bufs=1, space="PSUM") as pao_p:
            pao = pao_p.tile([128, 2, T, BL], f32)
            for m in range(2):
                for k in range(8):
                    nc.tensor.matmul(
                        pao[:, m], waot_s[:, k, m], h_t[:, k],
                        start=(k == 0), stop=(k == 7),
                    )
                nc.scalar.activation(hto[:, m], pao[:, m], AF.Identity,
                                     bias=bao_s[:, m:m + 1])
        outT = epool.tile([128, 2, D_OUT], f32, tag="outT")
        with tc.tile_pool(name="pout", bufs=1, space="PSUM") as pout_p:
            for m in range(2):
                po = pout_p.tile([128, 1024], f32, tag="po")
                for c0, c1 in ((0, 512), (512, D_OUT)):
                    for k in range(2):
                        nc.tensor.matmul(
                            po[:, c0:c1],
                            hto[:, k, m * 32:(m + 1) * 32], wot_s[:, k, c0:c1],
                            start=(k == 0), stop=False,
                        )
                    nc.tensor.matmul(po[:, c0:c1], on_s[:1], bo_s[:, c0:c1],
                                     start=False, stop=True)
                nc.scalar.activation(outT[:, m], po[:, :D_OUT], AF.Sigmoid)
                nc.sync.dma_start(out_d[m * 128:(m + 1) * 128], outT[:, m])

    nc.finalize()
    _NC_CACHE["nc"] = nc
    return nc


def kernel(**inputs):
    f32 = np.float32
    x = np.asarray(inputs["x"], f32)
    W_emb = np.asarray(inputs["W_emb"], f32)
    b_emb = np.asarray(inputs["b_emb"], f32)
    attn_w = np.asarray(inputs["attn_w"], f32)
    attn_b = float(np.asarray(inputs["attn_b"], f32))

    day_emb = (x.reshape(T * B, D_IN) @ W_emb.T + b_emb).reshape(T, B, D_DAY)
    fwd = _host_fwd(day_emb, inputs["Wih_f"], inputs["Whh_f"],
                    np.asarray(inputs["bih_f"], f32), np.asarray(inputs["bhh_f"], f32))
    w_f, w_r = attn_w[:H], attn_w[H:]
    fsc = fwd @ w_f + attn_b  # [T, B], attn_b folded in


    Wih_r = np.asarray(inputs["Wih_r"], f32)
    Whh_r = np.asarray(inputs["Whh_r"], f32)
    bih_r = np.asarray(inputs["bih_r"], f32)
    bhh_r = np.asarray(inputs["bhh_r"], f32)
    W_ao = np.asarray(inputs["W_ao"], f32)
    b_ao = np.asarray(inputs["b_ao"], f32)
    W_o = np.asarray(inputs["W_o"], f32)
    b_o = np.asarray(inputs["b_o"], f32)

    # shared (replicated) tensors
    wih_h = np.ascontiguousarray(
        Wih_r.reshape(6, 128, 2, 128).transpose(3, 2, 0, 1)).astype(BF16)
    whh_h = np.ascontiguousarray(
        Whh_r.reshape(6, 128, 2, 128).transpose(3, 2, 0, 1)).astype(BF16)
    wr_h = _fold2(w_r).astype(BF16)
    ones_h = np.ones((1, 128), BF16)
    brz_h = np.ascontiguousarray(
        (bih_r[:512] + bhh_r[:512]).reshape(4, 128).T).astype(f32)
    bnih_h = _fold2(bih_r[512:]).astype(f32)
    bnhh_h = _fold2(bhh_r[512:]).astype(f32)
    waot_h = np.ascontiguousarray(
        W_ao.reshape(2, 128, 8, 128).transpose(3, 2, 0, 1)).astype(BF16)
    bao_h = _fold2(b_ao).astype(f32)
    wot_h = np.ascontiguousarray(
        W_o.reshape(D_OUT, 2, 128).transpose(2, 1, 0)).astype(BF16)
    bo_h = b_o.reshape(1, D_OUT).astype(BF16)
    icnt_h = np.broadcast_to(
        (1.0 / np.arange(1, T + 1, dtype=f32))[:, None], (T, BL)
    ).reshape(1, T, BL).astype(f32)

    embT_all = np.ascontiguousarray(
        day_emb.transpose(2, 0, 1).reshape(2, 128, T, B).transpose(1, 0, 2, 3)
    ).astype(BF16)  # [128, 2, T, B]
    fwdT_all = np.ascontiguousarray(
        fwd.transpose(2, 0, 1).reshape(2, 128, T, B).transpose(1, 0, 2, 3)
    ).astype(BF16)

    in_maps = []
    for c in range(NC):
        bs = slice(c * BL, (c + 1) * BL)
        in_maps.append({
            "embT": np.ascontiguousarray(embT_all[:, :, :, bs]),
            "fwdT": np.ascontiguousarray(fwdT_all[:, :, :, bs]),
            "fvec": np.ascontiguousarray(fsc[:, bs]).reshape(1, T, BL).astype(f32),
        })

    consts = {
        "wih": wih_h, "whh": whh_h, "wr": wr_h, "ones1": ones_h,
        "brz": brz_h, "bnih": bnih_h, "bnhh": bnhh_h,
        "waot": waot_h, "bao": bao_h, "wot": wot_h, "bo": bo_h,
        "icnt": icnt_h,
    }
    nc = _build_nc(consts)
    from concourse.bass_utils import run_bass_kernel_spmd
    res = run_bass_kernel_spmd(nc, in_maps, core_ids=list(range(NC)))

    out = np.empty((T, B, D_OUT), f32)
    for c in range(NC):
        out[:, c * BL:(c + 1) * BL, :] = res.results[c]["out"].reshape(T, BL, D_OUT)
    return out


# revision 15
# speedup vs baseline: 1.2815x; 1.1066x over previous
import sys

sys.path.insert(0, "/opt/trn_rl_repo")

import numpy as np
import ml_dtypes

BF16 = ml_dtypes.bfloat16
T, B, D_IN, D_DAY, H, D_OUT = 64, 32, 4096, 256, 256, 942
NC = 8
BL = B // NC          # batch lanes per core
NB = T * BL           # (row, b) pairs per core = 256


def _sigmoid(x):
    with np.errstate(over="ignore"):
        return 1.0 / (1.0 + np.exp(-x))


def _host_fwd(day_emb, Wih, Whh, bih, bhh):
    # full forward GRU on host (tiny): day_emb [T,B,D_DAY] -> fwd [T,B,H]
    f32 = np.float32
    gi = day_emb.reshape(T * B, D_DAY) @ Wih.T.astype(f32) + bih
    gi = gi.reshape(T, B, 3 * H)
    WhhT = Whh.T.astype(f32)
    fwd = np.empty((T, B, H), f32)
    h = np.zeros((B, H), f32)
    for t in range(T):
        gh = h @ WhhT + bhh
        ir, iz, inn = gi[t, :, :H], gi[t, :, H:2 * H], gi[t, :, 2 * H:]
        hr, hz, hn = gh[:, :H], gh[:, H:2 * H], gh[:, 2 * H:]
        r = _sigmoid(ir + hr)
        z = _sigmoid(iz + hz)
        n = np.tanh(inn + r * hn)
        h = (1.0 - z) * n + z * h
        fwd[t] = h
    return fwd


def _fold2(v):  # [256] -> [128, 2]  (p, k) with feature = k*128+p
    return np.ascontiguousarray(v.reshape(2, 128).T)


_NC_CACHE = {}


def _build_nc(consts):
    if "nc" in _NC_CACHE:
        return _NC_CACHE["nc"]
    from contextlib import ExitStack
    import concourse.bass as bass
    from concourse import bacc, mybir
    from concourse.tile import TileContext

    bf = mybir.dt.bfloat16
    f32 = mybir.dt.float32
    AF = mybir.ActivationFunctionType
    ALU = mybir.AluOpType

    nc = bacc.Bacc("TRN2", num_devices=NC, debug=False, enable_asserts=False)

    def din(name, shape, dt):
        return nc.dram_tensor(name, shape, dt, kind="ExternalInput").ap()

    embT = din("embT", [128, 2, T, BL], bf)
    fwdT = din("fwdT", [128, 2, T, BL], bf)
    fvec = din("fvec", [1, T, BL], f32)

    def cin(name):
        return nc.inline_tensor(consts[name], name=name).ap()

    wih, whh, wr, ones1 = cin("wih"), cin("whh"), cin("wr"), cin("ones1")
    brz, bnih, bnhh = cin("brz"), cin("bnih"), cin("bnhh")
    waot, bao, wot, bo, icnt = (cin("waot"), cin("bao"), cin("wot"),
                                cin("bo"), cin("icnt"))
    out_d = nc.dram_tensor("out", [NB, D_OUT], bf, kind="ExternalOutput").ap()

    with TileContext(nc) as tc, ExitStack() as ctx:
        cpool = ctx.enter_context(tc.tile_pool(name="consts", bufs=1))

        def load(ap_dram, shape, dt):
            t = cpool.tile(shape, dt, tag=ap_dram.tensor.name)
            nc.sync.dma_start(t[:], ap_dram)
            return t

        emb_s = load(embT, [128, 2, T, BL], bf)
        fwd_s = load(fwdT, [128, 2, T, BL], bf)
        wih_s = load(wih, [128, 2, 6, 128], bf)
        whh_s = load(whh, [128, 2, 6, 128], bf)
        wr_s = load(wr, [128, 2], bf)
        on_s = load(ones1, [1, 128], bf)
        fv_s = load(fvec, [1, T, BL], f32)
        brz_s = load(brz, [128, 4], f32)
        bnih_s = load(bnih, [128, 2], f32)
        bnhh_s = load(bnhh, [128, 2], f32)
        waot_s = load(waot, [128, 8, 2, 128], bf)
        bao_s = load(bao, [128, 2], f32)
        wot_s = load(wot, [128, 2, D_OUT], bf)
        bo_s = load(bo, [1, D_OUT], bf)
        icnt_s = load(icnt, [1, T, BL], f32)

        # persistent state / accumulators
        spool = ctx.enter_context(tc.tile_pool(name="state", bufs=1))
        state = spool.tile([128, 2, T, BL], bf, tag="state")
        gix = spool.tile([128, 6, T, BL], bf, tag="gix")
        accR = spool.tile([128, 2, T, BL], f32, tag="accR")
        accF = spool.tile([128, 2, T, BL], f32, tag="accF")
        dacc = spool.tile([1, T, BL], f32, tag="dacc")
        h_t = spool.tile([128, 8, T, BL], bf, tag="h_t")

        nc.vector.memset(state[:], 0.0)
        nc.vector.memset(accR[:], 0.0)
        nc.vector.memset(accF[:], 0.0)
        nc.vector.memset(dacc[:], 0.0)

        # ---- bulk input projection: gix = Wih_r @ embT (+bih for n gates) ----
        with tc.tile_pool(name="pgix", bufs=1, space="PSUM") as pgix:
            pg = pgix.tile([128, 6, T, BL], f32)
            for m in range(6):
                for k in range(2):
                    nc.tensor.matmul(
                        pg[:, m], wih_s[:, k, m], emb_s[:, k],
                        start=(k == 0), stop=(k == 1),
                    )
            # rz gates: no bias here (added at sigmoid); n gates: + bih_n
            nc.scalar.activation(gix[:, 0:4], pg[:, 0:4], AF.Identity)
            for k in range(2):
                nc.scalar.activation(
                    gix[:, 4 + k], pg[:, 4 + k], AF.Identity,
                    bias=bnih_s[:, k:k + 1],
                )

        # ---- reverse recurrence with online attention ----
        lpool = ctx.enter_context(tc.tile_pool(name="loop", bufs=2))
        lctx = ctx.enter_context(ExitStack())
        prz_p = lctx.enter_context(tc.tile_pool(name="prz", bufs=1, space="PSUM"))
        phn_p = lctx.enter_context(tc.tile_pool(name="phn", bufs=1, space="PSUM"))
        ps_p = lctx.enter_context(tc.tile_pool(name="ps", bufs=2, space="PSUM"))
        ppb_p = lctx.enter_context(tc.tile_pool(name="ppb", bufs=2, space="PSUM"))

        for j in range(T):
            nact = T - j
            prz = prz_p.tile([128, 4, T, BL], f32, tag="prz")
            phn = phn_p.tile([128, 2, T, BL], f32, tag="phn")
            for m in range(4):
                for k in range(2):
                    nc.tensor.matmul(
                        prz[:, m, :nact], whh_s[:, k, m], state[:, k, j:],
                        start=(k == 0), stop=(k == 1),
                    )
            for m in range(2):
                for k in range(2):
                    nc.tensor.matmul(
                        phn[:, m, :nact], whh_s[:, k, 4 + m], state[:, k, j:],
                        start=(k == 0), stop=(k == 1),
                    )
            trz = lpool.tile([128, 4, T, BL], f32, tag="trz")
            nc.vector.tensor_add(trz[:, :, :nact], prz[:, :, :nact], gix[:, 0:4, :nact])
            rz = lpool.tile([128, 4, T, BL], bf, tag="rz")
            for m in range(4):
                nc.scalar.activation(
                    rz[:, m, :nact], trz[:, m, :nact], AF.Sigmoid,
                    bias=brz_s[:, m:m + 1],
                )
            # rn = (hn + bhh_n) * r  per fold
            rn = lpool.tile([128, 2, T, BL], f32, tag="rn")
            for k in range(2):
                nc.vector.scalar_tensor_tensor(
                    rn[:, k, :nact], phn[:, k, :nact], bnhh_s[:, k:k + 1],
                    rz[:, k, :nact], ALU.add, ALU.mult,
                )
            narg = lpool.tile([128, 2, T, BL], f32, tag="narg")
            nc.vector.tensor_add(narg[:, :, :nact], rn[:, :, :nact], gix[:, 4:6, :nact])
            n_g = lpool.tile([128, 2, T, BL], bf, tag="n_g")
            nc.scalar.activation(n_g[:, :, :nact], narg[:, :, :nact], AF.Tanh)
            dmn = lpool.tile([128, 2, T, BL], bf, tag="dmn")
            nc.vector.tensor_sub(dmn[:, :, :nact], state[:, :, j:], n_g[:, :, :nact])
            ezd = lpool.tile([128, 2, T, BL], bf, tag="ezd")
            nc.vector.tensor_mul(ezd[:, :, :nact], rz[:, 2:4, :nact], dmn[:, :, :nact])
            nc.vector.tensor_add(state[:, :, j:], n_g[:, :, :nact], ezd[:, :, :nact])

            # attention: s = w_r . state_new + f_j + attn_b ; p = exp(s)
            ps = ps_p.tile([1, T, BL], f32, tag="ps")
            for k in range(2):
                nc.tensor.matmul(
                    ps[:, :nact], wr_s[:, k:k + 1], state[:, k, j:],
                    start=(k == 0), stop=(k == 1),
                )
            s_f = lpool.tile([1, T, BL], f32, tag="s_f")
            nc.vector.tensor_add(
                s_f[:, :nact], ps[:, :nact],
                fv_s[:, j:j + 1].broadcast_to([1, nact, BL]),
            )
            p_b = lpool.tile([1, T, BL], bf, tag="p_b")
            nc.scalar.activation(p_b[:, :nact], s_f[:, :nact], AF.Exp)
            nc.vector.tensor_add(dacc[:, j:], dacc[:, j:], p_b[:, :nact])
            ppb = ppb_p.tile([128, T, BL], f32, tag="ppb")
            nc.tensor.matmul(ppb[:, :nact], on_s[:1], p_b[:, :nact],
                             start=True, stop=True)
            tmr = lpool.tile([128, 2, T, BL], f32, tag="tmr")
            tmf = lpool.tile([128, 2, T, BL], f32, tag="tmf")
            for k in range(2):
                nc.vector.tensor_mul(tmr[:, k, :nact], ppb[:, :nact], state[:, k, j:])
                nc.vector.tensor_add(accR[:, k, j:], accR[:, k, j:], tmr[:, k, :nact])
                nc.vector.tensor_mul(
                    tmf[:, k, :nact], ppb[:, :nact],
                    fwd_s[:, k, j:j + 1].broadcast_to([128, nact, BL]),
                )
                nc.vector.tensor_add(accF[:, k, j:], accF[:, k, j:], tmf[:, k, :nact])

        # ---- epilogue: normalize, assemble h_t, two projections, DMA out ----
        lctx.close()
        epool = ctx.enter_context(tc.tile_pool(name="epi", bufs=1))
        invd = epool.tile([1, T, BL], f32, tag="invd")
        nc.vector.reciprocal(invd[:], dacc[:])
        scal = epool.tile([1, T, BL], bf, tag="scal")
        nc.vector.tensor_mul(scal[:], invd[:], icnt_s[:])
        with tc.tile_pool(name="psc", bufs=1, space="PSUM") as psc_p:
            psc = psc_p.tile([128, T, BL], f32)
            nc.tensor.matmul(psc[:], on_s[:1], scal[:], start=True, stop=True)
            for k in range(2):
                nc.vector.tensor_mul(h_t[:, 0 + k], accF[:, k], psc[:])
                nc.vector.tensor_mul(h_t[:, 2 + k], accR[:, k], psc[:])
                nc.vector.tensor_copy(h_t[:, 4 + k], fwd_s[:, k])
                nc.vector.tensor_copy(h_t[:, 6 + k], state[:, k])
        hto = epool.tile([128, 2, T, BL], bf, tag="hto")
        with tc.tile_pool(name="pao", bufs=1, space="PSUM") as pao_p:
            pao = pao_p.tile([128, 2, T, BL], f32)
            for m in range(2):
                for k in range(8):
                    nc.tensor.matmul(
                        pao[:, m], waot_s[:, k, m], h_t[:, k],
                        start=(k == 0), stop=(k == 7),
                    )
                nc.scalar.activation(hto[:, m], pao[:, m], AF.Identity,
                                     bias=bao_s[:, m:m + 1])
        outT = epool.tile([128, 2, D_OUT], bf, tag="outT")
        with tc.tile_pool(name="pout", bufs=1, space="PSUM") as pout_p:
            for m in range(2):
                po = pout_p.tile([128, 1024], f32, tag="po")
                for c0, c1 in ((0, 512), (512, D_OUT)):
                    for k in range(2):
                        nc.tensor.matmul(
                            po[:, c0:c1],
                            hto[:, k, m * 32:(m + 1) * 32], wot_s[:, k, c0:c1],
                            start=(k == 0), stop=False,
                        )
                    nc.tensor.matmul(po[:, c0:c1], on_s[:1], bo_s[:, c0:c1],
                                     start=False, stop=True)
                nc.scalar.activation(outT[:, m], po[:, :D_OUT], AF.Sigmoid)
                nc.sync.dma_start(out_d[m * 128:(m + 1) * 128], outT[:, m])

    nc.finalize()
    _NC_CACHE["nc"] = nc
    return nc


def kernel(**inputs):
    f32 = np.float32
    x = np.asarray(inputs["x"], f32)
    W_emb = np.asarray(inputs["W_emb"], f32)
    b_emb = np.asarray(inputs["b_emb"], f32)
    attn_w = np.asarray(inputs["attn_w"], f32)
    attn_b = float(np.asarray(inputs["attn_b"], f32))

    day_emb = (x.reshape(T * B, D_IN) @ W_emb.T + b_emb).reshape(T, B, D_DAY)
    fwd = _host_fwd(day_emb, inputs["Wih_f"], inputs["Whh_f"],
                    np.asarray(inputs["bih_f"], f32), np.asarray(inputs["bhh_f"], f32))
    w_f, w_r = attn_w[:H], attn_w[H:]
    fsc = fwd @ w_f + attn_b  # [T, B], attn_b folded in


    Wih_r = np.asarray(inputs["Wih_r"], f32)
    Whh_r = np.asarray(inputs["Whh_r"], f32)
    bih_r = np.asarray(inputs["bih_r"], f32)
    bhh_r = np.asarray(inputs["bhh_r"], f32)
    W_ao = np.asarray(inputs["W_ao"], f32)
    b_ao = np.asarray(inputs["b_ao"], f32)
    W_o = np.asarray(inputs["W_o"], f32)
    b_o = np.asarray(inputs["b_o"], f32)

    # shared (replicated) tensors
    wih_h = np.ascontiguousarray(
        Wih_r.reshape(6, 128, 2, 128).transpose(3, 2, 0, 1)).astype(BF16)
    whh_h = np.ascontiguousarray(
        Whh_r.reshape(6, 128, 2, 128).transpose(3, 2, 0, 1)).astype(BF16)
    wr_h = _fold2(w_r).astype(BF16)
    ones_h = np.ones((1, 128), BF16)
    brz_h = np.ascontiguousarray(
        (bih_r[:512] + bhh_r[:512]).reshape(4, 128).T).astype(f32)
    bnih_h = _fold2(bih_r[512:]).astype(f32)
    bnhh_h = _fold2(bhh_r[512:]).astype(f32)
    waot_h = np.ascontiguousarray(
        W_ao.reshape(2, 128, 8, 128).transpose(3, 2, 0, 1)).astype(BF16)
    bao_h = _fold2(b_ao).astype(f32)
    wot_h = np.ascontiguousarray(
        W_o.reshape(D_OUT, 2, 128).transpose(2, 1, 0)).astype(BF16)
    bo_h = b_o.reshape(1, D_OUT).astype(BF16)
    icnt_h = np.broadcast_to(
        (1.0 / np.arange(1, T + 1, dtype=f32))[:, None], (T, BL)
    ).reshape(1, T, BL).astype(f32)

    embT_all = np.ascontiguousarray(
        day_emb.transpose(2, 0, 1).reshape(2, 128, T, B).transpose(1, 0, 2, 3)
    ).astype(BF16)  # [128, 2, T, B]
    fwdT_all = np.ascontiguousarray(
        fwd.transpose(2, 0, 1).reshape(2, 128, T, B).transpose(1, 0, 2, 3)
    ).astype(BF16)

    in_maps = []
    for c in range(NC):
        bs = slice(c * BL, (c + 1) * BL)
        in_maps.append({
            "embT": np.ascontiguousarray(embT_all[:, :, :, bs]),
            "fwdT": np.ascontiguousarray(fwdT_all[:, :, :, bs]),
            "fvec": np.ascontiguousarray(fsc[:, bs]).reshape(1, T, BL).astype(f32),
        })

    consts = {
        "wih": wih_h, "whh": whh_h, "wr": wr_h, "ones1": ones_h,
        "brz": brz_h, "bnih": bnih_h, "bnhh": bnhh_h,
        "waot": waot_h, "bao": bao_h, "wot": wot_h, "bo": bo_h,
        "icnt": icnt_h,
    }
    nc = _build_nc(consts)
    from concourse.bass_utils import run_bass_kernel_spmd
    res = run_bass_kernel_spmd(nc, in_maps, core_ids=list(range(NC)))

    out = np.empty((T, B, D_OUT), f32)
    for c in range(NC):
        out[:, c * BL:(c + 1) * BL, :] = np.asarray(res.results[c]["out"]).astype(f32).reshape(T, BL, D_OUT)
    return out
